# revision 25
# baseline (speedup 1.0000x reference)
"""Trainium2 Bass kernel for nn_DeltaNet_22488448762128 (v3).

Full-input contract: kernel(**inputs) takes the unsharded numpy inputs and
returns the full [B, L, HID] output. Internally shards across 8 NeuronCores:
core = (b, hg) with b in {0,1} and hg in {0..3} head-groups of 4 heads.
Each core computes projections for its 4 heads, a chunked (C=128) linear
attention scan, and a partial output projection; the host sums the 4 partial
outputs per batch element and adds bo.

Math (per head, chunk c of size C, state S aug with z column):
  a_t   = cumprod(beta) within chunk;  aC = a_{C-1}
  q~_t  = phi(rope(q))_t * a_t ;  k^_s = phi(rope(k))_s * aC / a_s
  A^T[s,t] = (phi_k_s . q~_t) * (1/a_s) * [s<=t]
  nu    = A^T.T @ [V|1] + q~ @ S_aug   ;  y_t = nu[:, :D] / (nu[:, D] + eps)
  S_aug = aC * S_aug + k^T @ [V|1]

Precision: projections and the output projection run on the PE in fp8 (e4m3)
DoubleRow mode with a 3-term error-compensated decomposition
  x @ W  =  x8 @ W8  +  xl8 @ W8'  +  x8 @ Wl8
(x8 = fp8(x), xl8 = fp8(16*(x - x8)), Wl = W - fp8(32W)/32), recovering
~bf16-class accuracy at 0.75x the bf16 PE cost (DoubleRow pairs two 128-row
contraction tiles per pass). Weights are pre-scaled by 32 into e4m3's normal
range; the inverse scale is folded into the rope tables, the sigmoid's
activation scale, the augmented-ones column (=32), eps (*32), and the final
output-copy scale. The scan runs in bf16 with fp32 PSUM accumulation; the
rope/phi elementwise path is bf16 end-to-end (2x DVE).

PSUM banks (8): pj[big x2] g/q/k/v projection groups; ppo[po x2] output
projection; ptp[tp3 x2] per-head q/k/y transposes; pnu[nuA x2] beta-chain
transposes + per-head A|nu|U groups.

Engine split: PE matmuls; ACT copies + exps; DVE rope/phi/masks/y; GPSIMD
phi-mins, v-copy, S-update.
"""

import math
import numpy as np
import ml_dtypes

B, L, HID = 2, 2048, 2048
H, D = 16, 128
HG = 4              # heads per core
C = 128             # chunk size
NCHUNK = L // C     # 16
NK = HID // C       # 16 contraction tiles
NKP = NK // 2       # 8 DoubleRow pair-tiles
EPS = 1e-6
BETA_MIN, BETA_MAX = 0.8, 0.9995
NCORES = 8
GW = HG * D         # 512, per-core projection width
NO = HID // GW      # 4 output col tiles
SCALE = 32.0        # fp8 weight pre-scale
XL_S = 16.0         # x residual pre-scale
E4 = ml_dtypes.float8_e4m3

# fp8 GEMM term counts: 2 = x8@W8 + xl8@W8' (x-quantization compensated),
# 3 = + x8@Wl8 (weight-quantization compensated too). g is ~free (N=4) so it
# keeps 3 terms; q/k/v tolerate W-quant noise (it largely cancels or averages
# out through the normalized attention).
WTERMS = {"q": 2, "k": 2, "v": 3, "g": 3}

_CACHE = {}


def _rope_tables():
    half = D // 2
    inv_freq = (1.0 / (10000.0 ** (np.arange(half, dtype=np.float32) /
                                   np.float32(half)))).astype(np.float32)
    t = np.arange(L, dtype=np.float32)
    freqs = t[:, None] * inv_freq[None, :]
    # fold the fp8 weight pre-scale out of q/k here: tables are cos/32, sin/32
    cos = (np.cos(freqs) / SCALE).astype(ml_dtypes.bfloat16)   # [L, 64]
    sin = (np.sin(freqs) / SCALE).astype(ml_dtypes.bfloat16)
    # chunk-major: [128, NCHUNK*64], block c = rows c*128..c*128+128
    def rearr(m):
        return np.ascontiguousarray(
            m.reshape(NCHUNK, C, half).transpose(1, 0, 2).reshape(C, NCHUNK * half))
    return rearr(cos), rearr(sin)


def _build(cfg):
    import concourse.bass as bass
    import concourse.bacc as bacc
    import concourse.tile as tile
    import concourse.mybir as mybir
    from contextlib import ExitStack

    dt = mybir.dt
    F32 = dt.float32
    BF16 = dt.bfloat16
    F8 = dt.float8e4
    DRm = mybir.MatmulPerfMode.DoubleRow
    Alu = mybir.AluOpType
    Act = mybir.ActivationFunctionType
    half = D // 2

    nch = cfg.get("nchunk", NCHUNK)

    nc = bacc.Bacc("TRN2", target_bir_lowering=False, debug=False,
                   enable_asserts=False, num_devices=NCORES)

    # ---- DRAM I/O (host passes PE-blocked layouts, see make_in_maps) ----
    xT_d = nc.dram_tensor("xTb", [NCHUNK, C, 2 * HID], F8, kind="ExternalInput").ap()
    wq_d = nc.dram_tensor("wq3", [C, WTERMS["q"] * NK * GW], F8,
                          kind="ExternalInput").ap()
    wk_d = nc.dram_tensor("wk3", [C, WTERMS["k"] * NK * GW], F8,
                          kind="ExternalInput").ap()
    wv_d = nc.dram_tensor("wv3", [C, WTERMS["v"] * NK * GW], F8,
                          kind="ExternalInput").ap()
    wg_d = nc.dram_tensor("wg3", [C, WTERMS["g"] * NK * HG], F8,
                          kind="ExternalInput").ap()
    wo_d = nc.dram_tensor("wo3", [C, 2 * HG * HID], F8, kind="ExternalInput").ap()
    nbg_d = nc.dram_tensor("nbg4", [C, HG], F32, kind="ExternalInput").ap()
    cos_d = nc.dram_tensor("cosr", [C, NCHUNK * half], BF16, kind="ExternalInput").ap()
    sin_d = nc.dram_tensor("sinr", [C, NCHUNK * half], BF16, kind="ExternalInput").ap()
    mask_d = nc.dram_tensor("maskT", [C, C], F32, kind="ExternalInput").ap()
    id_d = nc.dram_tensor("ident", [C, C], F32, kind="ExternalInput").ap()
    out_d = nc.dram_tensor("out", [L, HID], BF16, kind="ExternalOutput").ap()

    def pair(t, off, step, f):
        b = t[:]
        return bass.AP(tensor=b.tensor, offset=b.offset + off,
                       ap=[b.ap[0], [step, 2], [1, f]])

    with ExitStack() as ctx:
        tc = ctx.enter_context(tile.TileContext(nc))

        cpool = ctx.enter_context(tc.tile_pool(name="consts", bufs=1))
        cos_t = cpool.tile([C, NCHUNK * half], BF16, tag="cos")
        sin_t = cpool.tile([C, NCHUNK * half], BF16, tag="sin")
        mask_t = cpool.tile([C, C], F32, tag="mask")
        id_t = cpool.tile([C, C], F32, tag="id")
        id_s = cpool.tile([C, C], BF16, tag="id_s")
        ones_t = cpool.tile([C, C], F32, tag="ones")
        nbg_t = cpool.tile([C, HG], F32, tag="nbg")
        nc.vector.memset(ones_t[:], 1.0)

        with ExitStack() as main:
            wpool = main.enter_context(tc.tile_pool(name="w", bufs=1))
            wq_t = wpool.tile([C, WTERMS["q"] * NK * GW], F8, tag="wq")
            wk_t = wpool.tile([C, WTERMS["k"] * NK * GW], F8, tag="wk")
            wv_t = wpool.tile([C, WTERMS["v"] * NK * GW], F8, tag="wv")
            wg_t = wpool.tile([C, WTERMS["g"] * NK * HG], F8, tag="wg")
            wo_t = wpool.tile([C, 2 * HG * HID], F8, tag="wo")
            nc.sync.dma_start(wg_t[:], wg_d)
            nc.sync.dma_start(cos_t[:], cos_d)
            nc.sync.dma_start(sin_t[:], sin_d)
            nc.sync.dma_start(mask_t[:], mask_d)
            nc.sync.dma_start(id_t[:], id_d)
            nc.sync.dma_start(nbg_t[:], nbg_d)

            # chunk-local SBUF pools
            xp = main.enter_context(tc.tile_pool(name="xp", bufs=cfg.get("xp", 2)))

            # prefetch the first two x chunks ahead of the weight stream so
            # chunk 0's projections aren't queued behind 11 MB of weights
            xpre = []
            for cpre in range(min(2, nch)):
                t = xp.tile([C, 2 * HID], F8, tag="xtb")
                nc.scalar.dma_start(t[:], xT_d[cpre])
                xpre.append(t)

            # weights streamed in PE consumption order (g,q,k,v then wo),
            # sliced so the PE can trail the DMA k-pair by k-pair
            TW = NK * GW
            for w_t, w_d, nt in ((wq_t, wq_d, WTERMS["q"]),
                                 (wk_t, wk_d, WTERMS["k"]),
                                 (wv_t, wv_d, WTERMS["v"])):
                for term in range(nt):
                    for hf in range(2):
                        sl = slice(term * TW + hf * TW // 2,
                                   term * TW + (hf + 1) * TW // 2)
                        nc.sync.dma_start(w_t[:, sl], w_d[:, sl])
            for term in (0, 1):
                ts_ = bass.ts(term, HG * HID)
                nc.sync.dma_start(wo_t[:, ts_], wo_d[:, ts_])
            nc.scalar.copy(id_s[:], id_t[:])
            big2 = main.enter_context(tc.tile_pool(name="big2", bufs=2))
            sml = main.enter_context(tc.tile_pool(name="sml", bufs=cfg.get("sml", 4)))
            spool = main.enter_context(tc.tile_pool(name="spool", bufs=2))
            ypool = main.enter_context(tc.tile_pool(name="ypool", bufs=2))
            osb = main.enter_context(tc.tile_pool(name="osb", bufs=cfg.get("osb", 4)))

            # psum pools: pj 2 + ppo 2 + ptp 2 + pnu 2 = 8 banks
            pj = main.enter_context(tc.tile_pool(
                name="pj", bufs=cfg.get("pj", 2), space="PSUM"))
            ppo = main.enter_context(tc.tile_pool(
                name="ppo", bufs=cfg.get("ppo", 2), space="PSUM"))
            ptp = main.enter_context(tc.tile_pool(
                name="ptp", bufs=cfg.get("ptp", 2), space="PSUM"))
            pnu = main.enter_context(tc.tile_pool(
                name="pnu", bufs=cfg.get("pnu", 2), space="PSUM"))

            S_cur = []
            for h in range(HG):
                s0 = spool.tile([C, D + 1], BF16, tag=f"s{h}")
                nc.vector.memset(s0[:], 0.0)
                S_cur.append(s0)

            def proj_mms(ps, fw, w_t, tw, nterm):
                n = 0
                for term in range(nterm):
                    xoff = HID if term == 1 else 0
                    for kp in range(NKP):
                        nc.tensor.matmul(
                            ps[:, 0:fw],
                            pair(xtb, xoff + kp * 2 * C, C, C),
                            pair(w_t, term * tw + kp * 2 * fw, fw, fw),
                            start=(n == 0), stop=(n == nterm * NKP - 1),
                            perf_mode=DRm)
                        n += 1

            def rope(src, dst, tmp, tmp2=None):
                # tmp2 set: de-half on DVE, do-half on GPSIMD concurrently
                ed = nc.vector
                eo = nc.gpsimd if tmp2 is not None else nc.vector
                se = src[:].rearrange("p (h d) -> p h d", h=HG)[:, :, 0:half]
                so = src[:].rearrange("p (h d) -> p h d", h=HG)[:, :, half:D]
                de = dst[:].rearrange("p (h d) -> p h d", h=HG)[:, :, 0:half]
                do = dst[:].rearrange("p (h d) -> p h d", h=HG)[:, :, half:D]
                cc = bass.AP(tensor=cos_t[:].tensor,
                             offset=cos_t[:, bass.ts(c, half)].offset,
                             ap=[cos_t[:].ap[0], [0, HG], [1, half]])
                ss = bass.AP(tensor=sin_t[:].tensor,
                             offset=sin_t[:, bass.ts(c, half)].offset,
                             ap=[sin_t[:].ap[0], [0, HG], [1, half]])
                t1 = tmp[:].rearrange("p (h d) -> p h d", h=HG)[:, :, 0:half]
                t2 = tmp[:].rearrange("p (h d) -> p h d", h=HG)[:, :, half:D]
                tb = tmp2 if tmp2 is not None else tmp
                t3 = tb[:].rearrange("p (h d) -> p h d", h=HG)[:, :, 0:half]
                t4 = tb[:].rearrange("p (h d) -> p h d", h=HG)[:, :, half:D]
                ed.tensor_tensor(out=t1, in0=se, in1=cc, op=Alu.mult)
                ed.tensor_tensor(out=t2, in0=so, in1=ss, op=Alu.mult)
                ed.tensor_tensor(out=de, in0=t1, in1=t2, op=Alu.subtract)
                eo.tensor_tensor(out=t3, in0=se, in1=ss, op=Alu.mult)
                eo.tensor_tensor(out=t4, in0=so, in1=cc, op=Alu.mult)
                eo.tensor_tensor(out=do, in0=t3, in1=t4, op=Alu.add)

            po_prev = None  # (yt8, ytl) of previous chunk

            def emit_po(ysrcs, c_out, orange=None):
                yt8_, ytl_ = ysrcs
                for o in (orange if orange is not None else range(NO)):
                    out_ps = ppo.tile([C, GW], F32, tag="po")
                    n = 0
                    for slot, ysrc in ((0, yt8_), (0, ytl_), (1, yt8_)):
                        for hp in range(HG // 2):
                            nc.tensor.matmul(
                                out_ps[:],
                                pair(ysrc, hp * 2 * C, C, C),
                                pair(wo_t,
                                     slot * HG * HID + (2 * hp) * HID + o * GW,
                                     HID, GW),
                                start=(n == 0), stop=(n == 5),
                                perf_mode=DRm)
                            n += 1
                    out_sb = osb.tile([C, GW], BF16, tag="osb")
                    nc.scalar.mul(out_sb[:], out_ps[:], 1.0 / SCALE)
                    nc.sync.dma_start(out_d[bass.ts(c_out, C), bass.ts(o, GW)],
                                      out_sb[:])

            for c in range(nch):
                if c < len(xpre):
                    xtb = xpre[c]
                else:
                    xtb = xp.tile([C, 2 * HID], F8, tag="xtb")
                    nc.scalar.dma_start(xtb[:], xT_d[c])

                # ---- projections (PE) interleaved with beta chain ----
                # g first so the long beta dependency chain starts early
                g_ps = pj.tile([C, GW], F32, tag="big")
                proj_mms(g_ps, HG, wg_t, NK * HG, WTERMS["g"])
                beta_sb = sml.tile([C, HG], F32, tag="beta")
                nc.scalar.activation(beta_sb[:], g_ps[:, 0:HG], Act.Exp,
                                     scale=-1.0 / SCALE)
                nc.vector.scalar_tensor_tensor(
                    out=beta_sb[:], in0=beta_sb[:], scalar=1.0,
                    in1=nbg_t[:], op0=Alu.mult, op1=Alu.mult)
                nc.vector.tensor_scalar_add(beta_sb[:], beta_sb[:], 1.0)
                nc.vector.reciprocal(beta_sb[:], beta_sb[:])
                nc.vector.tensor_scalar(out=beta_sb[:], in0=beta_sb[:],
                                        scalar1=BETA_MIN, scalar2=BETA_MAX,
                                        op0=Alu.max, op1=Alu.min)

                q_ps = pj.tile([C, GW], F32, tag="big")
                proj_mms(q_ps, GW, wq_t, TW, WTERMS["q"])
                q_sb = big2.tile([C, GW], BF16, tag="q")
                nc.scalar.copy(q_sb[:], q_ps[:])

                # beta transposes ride the pnu rotation between head uses
                btp_ps = pnu.tile([C, 3 * (D + 1) - 1], F32, tag="nuA")
                nc.tensor.transpose(btp_ps[0:HG, 0:C], beta_sb[:], id_t[:])
                btp_sb = sml.tile([HG, C], F32, tag="btp")
                nc.scalar.copy(btp_sb[:], btp_ps[0:HG, 0:C])
                aT_sb = sml.tile([HG, C], F32, tag="aT")
                nc.vector.tensor_tensor_scan(
                    out=aT_sb[:], data0=btp_sb[:], data1=ones_t[0:HG, :],
                    initial=1.0, op0=Alu.mult, op1=Alu.mult)

                k_ps = pj.tile([C, GW], F32, tag="big")
                proj_mms(k_ps, GW, wk_t, TW, WTERMS["k"])
                k_sb = big2.tile([C, GW], BF16, tag="k")
                nc.scalar.copy(k_sb[:], k_ps[:])

                a_ps = pnu.tile([C, 3 * (D + 1) - 1], F32, tag="nuA")
                nc.tensor.transpose(a_ps[:, 0:HG], aT_sb[:], id_t[0:HG, 0:HG])
                a_sb = sml.tile([C, HG], F32, tag="a")
                nc.scalar.copy(a_sb[:], a_ps[:, 0:HG])
                ainv_sb = sml.tile([C, HG], F32, tag="ainv")
                nc.vector.reciprocal(ainv_sb[:], a_sb[:])
                diag4 = sml.tile([HG, HG], F32, tag="diag4")
                nc.vector.tensor_scalar(out=diag4[:], in0=id_t[0:HG, 0:HG],
                                        scalar1=aT_sb[:, C - 1:C], scalar2=None,
                                        op0=Alu.mult)

                # rope(q) early on DVE; phi(q) min on GPSIMD
                qr = big2.tile([C, GW], BF16, tag="qr")
                rtq = big2.tile([C, GW], BF16, tag="rtq")
                rtq2 = big2.tile([C, GW], BF16, tag="rtq2")
                rope(q_sb, qr, rtq, rtq2)
                tmq = big2.tile([C, GW], BF16, tag="mq")
                nc.gpsimd.tensor_scalar_min(tmq[:], qr[:], 0.0)
                teq = big2.tile([C, GW], BF16, tag="eq")
                nc.scalar.activation(teq[:], tmq[:], Act.Exp)
                phiq = big2.tile([C, GW], BF16, tag="phq")
                nc.vector.scalar_tensor_tensor(out=phiq[:], in0=qr[:],
                                               scalar=0.0, in1=teq[:],
                                               op0=Alu.max, op1=Alu.add)

                v_ps = pj.tile([C, GW], F32, tag="big")
                proj_mms(v_ps, GW, wv_t, TW, WTERMS["v"])
                v_sb = big2.tile([C, HG * (D + 1)], BF16, tag="v")
                v_aug = v_sb[:].rearrange("p (h e) -> p h e", e=D + 1)
                nc.scalar.copy(v_aug[:, :, 0:D],
                               v_ps[:, 0:GW].rearrange("p (h e) -> p h e", e=D))
                nc.vector.memset(v_aug[:, :, D:D + 1], SCALE)

                acb_ps = pnu.tile([C, 3 * (D + 1) - 1], F32, tag="nuA")
                nc.tensor.matmul(acb_ps[:, 0:HG], ones_t[0:HG, :], diag4[:],
                                 start=True, stop=True)
                acb_sb = sml.tile([C, HG], F32, tag="acb")
                nc.scalar.copy(acb_sb[:], acb_ps[:, 0:HG])
                acdiv_sb = sml.tile([C, HG], F32, tag="acdiv")
                nc.vector.tensor_tensor(out=acdiv_sb[:], in0=ainv_sb[:],
                                        in1=acb_sb[:], op=Alu.mult)

                # rope(k) + phi(k)
                kr = big2.tile([C, GW], BF16, tag="kr")
                rtk = big2.tile([C, GW], BF16, tag="rtk")
                rope(k_sb, kr, rtk)
                tmk = big2.tile([C, GW], BF16, tag="mk")
                nc.gpsimd.tensor_scalar_min(tmk[:], kr[:], 0.0)
                tek = big2.tile([C, GW], BF16, tag="ek")
                nc.scalar.activation(tek[:], tmk[:], Act.Exp)
                phik = big2.tile([C, GW], BF16, tag="phk")
                nc.vector.scalar_tensor_tensor(out=phik[:], in0=kr[:],
                                               scalar=0.0, in1=tek[:],
                                               op0=Alu.max, op1=Alu.add)

                # ---- scan, 2-wide head pipeline ----
                yt8 = ypool.tile([C, HG * C], F8, tag="yt8")
                ytl = ypool.tile([C, HG * C], F8, tag="ytl")

                def head_tp(h):
                    hs = bass.ts(h, D)
                    tp = ptp.tile([C, 3 * D], BF16, tag="tp3")
                    nc.tensor.transpose(tp[:, 0:D], phiq[:, hs], id_s[:])
                    nc.tensor.transpose(tp[:, D:2 * D], phik[:, hs], id_s[:])
                    qkT = sml.tile([C, 2 * D], BF16, tag="qkT")
                    nc.scalar.copy(qkT[:], tp[:, 0:2 * D])
                    return tp, qkT

                def head_A(h, st):
                    tp, qkT = st
                    nuA = pnu.tile([C, 3 * (D + 1) - 1], F32, tag="nuA")
                    Ar = nuA[:, 2 * (D + 1):3 * (D + 1) - 1]
                    nc.tensor.matmul(Ar[:], qkT[:, D:2 * D], qkT[:, 0:D],
                                     start=True, stop=True)
                    A_sb = sml.tile([C, C], BF16, tag="A")
                    nc.vector.scalar_tensor_tensor(
                        out=A_sb[:], in0=Ar[:],
                        scalar=ainv_sb[:, h:h + 1], in1=mask_t[:],
                        op0=Alu.mult, op1=Alu.mult)
                    # khat after the Ar read on the in-order DVE: U's matmul
                    # (which waits on khat) then cannot re-open the bank's
                    # accumulation group before Ar is consumed
                    hs = bass.ts(h, D)
                    khat = sml.tile([C, D], BF16, tag="khat")
                    nc.vector.tensor_scalar_mul(khat[:], phik[:, hs],
                                                acdiv_sb[:, h:h + 1])
                    return (*st, nuA, A_sb, khat)

                def head_nu(h, st):
                    tp, qkT, nuA, A_sb, khat = st
                    nu = nuA[:, 0:D + 1]
                    U = nuA[:, D + 1:2 * (D + 1)]
                    vh = v_sb[:, h * (D + 1):(h + 1) * (D + 1)]
                    nc.tensor.matmul(U[:], khat[:], vh, start=True, stop=False)
                    nc.tensor.matmul(nu[:], A_sb[:], vh, start=False, stop=False)
                    nc.tensor.matmul(nu[:], qkT[:, 0:D], S_cur[h][:],
                                     start=False, stop=True)
                    S_new = spool.tile([C, D + 1], BF16, tag=f"s{h}")
                    nc.vector.scalar_tensor_tensor(
                        out=S_new[:], in0=S_cur[h][:], scalar=acb_sb[:, h:h + 1],
                        in1=U, op0=Alu.mult, op1=Alu.add)
                    S_cur[h] = S_new
                    # denom = phi_q . z-path is strictly positive (phi > 0),
                    # so the reference's +eps (1e-6, i.e. ~1e-8 relative) is
                    # dropped and the reciprocal reads PSUM directly
                    rd = sml.tile([C, 1], F32, tag="rd")
                    nc.vector.reciprocal(rd[:], nu[:, D:D + 1])
                    y_bf = sml.tile([C, D], BF16, tag="ybf")
                    nc.vector.tensor_scalar_mul(y_bf[:], nu[:, 0:D], rd[:])
                    return (*st, y_bf)

                def head_yT(h, st):
                    tp, y_bf = st[0], st[-1]
                    nc.tensor.transpose(tp[:, 2 * D:3 * D], y_bf[:], id_s[:])
                    ys = bass.ts(h, C)
                    nc.scalar.copy(yt8[:, ys], tp[:, 2 * D:3 * D])
                    nc.vector.tensor_tensor(out=ytl[:, ys], in0=tp[:, 2 * D:3 * D],
                                            in1=yt8[:, ys], op=Alu.subtract)

                # out-projection of the PREVIOUS chunk is interleaved into the
                # scan as PE filler work behind the DVE/ACT dependency chains
                for p in (0, 2):
                    s0 = head_tp(p)
                    s1 = head_tp(p + 1)
                    s0 = head_A(p, s0)
                    s1 = head_A(p + 1, s1)
                    s0 = head_nu(p, s0)
                    s1 = head_nu(p + 1, s1)
                    if po_prev is not None:
                        emit_po(po_prev[0], po_prev[1], (p, p + 1))
                    head_yT(p, s0)
                    head_yT(p + 1, s1)
                po_prev = ((yt8, ytl), c)

            emit_po(po_prev[0], po_prev[1])

    nc.compile()
    return nc


def _get_nc(cfg_key="default", **cfg):
    if cfg_key not in _CACHE:
        _CACHE[cfg_key] = _build(cfg)
    return _CACHE[cfg_key]


def _blk(m, fw):
    # [HID, fw] -> [C, NK*fw] with block k = m[k*128:(k+1)*128, :]
    return np.ascontiguousarray(
        m.reshape(NK, C, fw).transpose(1, 0, 2).reshape(C, NK * fw))


def _w3(W, fw, nterm):
    """W [HID, fw] f32 -> [C, nterm*NK*fw] e4m3: fp8(32W) | fp8(2W) | fp8(32Wl)."""
    W = np.asarray(W, np.float32)
    t0 = (SCALE * W).astype(E4)
    terms = [_blk(t0, fw), _blk((2.0 * W).astype(E4), fw)]
    if nterm == 3:
        wl = W - t0.astype(np.float32) / SCALE
        terms.append(_blk((SCALE * wl).astype(E4), fw))
    return np.ascontiguousarray(np.concatenate(terms, axis=1))


def _wo3(Wo):
    """Wo [GW, HID] f32 -> [C, 2*HG*HID] e4m3, blocked by head: fp8(32Wo)
    (shared by the y8 and y-residual terms, both at 32x scale) | fp8(32*Wol)."""
    Wo = np.asarray(Wo, np.float32)
    t0 = (SCALE * Wo).astype(E4)
    wl = Wo - t0.astype(np.float32) / SCALE
    t2 = (SCALE * wl).astype(E4)

    def blk(m):
        return m.reshape(HG, C, HID).transpose(1, 0, 2).reshape(C, HG * HID)

    return np.ascontiguousarray(np.concatenate([blk(t0), blk(t2)], axis=1))


def make_in_maps(x, Wq, Wk, Wv, Wg, bg, Wo, bo):
    cosr, sinr = _rope_tables()
    maskT = np.triu(np.ones((C, C), np.float32))
    ident = np.eye(C, dtype=np.float32)
    x = np.asarray(x, np.float32)
    Wq, Wk, Wv = np.asarray(Wq), np.asarray(Wk), np.asarray(Wv)
    Wg, bg, Wo = np.asarray(Wg), np.asarray(bg), np.asarray(Wo)
    in_maps = []
    xTb_cache = {}
    for core in range(NCORES):
        b, hg = divmod(core, 4)
        cs = slice(hg * GW, (hg + 1) * GW)
        hsl = slice(hg * HG, (hg + 1) * HG)
        if b not in xTb_cache:
            # xTb[c, p, k*128+f] = x[b][c*128+f, k*128+p]; fp8 + 16*residual
            xT = np.ascontiguousarray(
                x[b].reshape(NCHUNK, C, NK, C).transpose(0, 3, 2, 1)
                .reshape(NCHUNK, C, HID)).astype(np.float32)
            x8 = xT.astype(E4)
            xl8 = (XL_S * (xT - x8.astype(np.float32))).astype(E4)
            xTb_cache[b] = np.ascontiguousarray(
                np.concatenate([x8, xl8], axis=2))
        in_maps.append({
            "xTb": xTb_cache[b],
            "wq3": _w3(Wq[:, cs], GW, WTERMS["q"]),
            "wk3": _w3(Wk[:, cs], GW, WTERMS["k"]),
            "wv3": _w3(Wv[:, cs], GW, WTERMS["v"]),
            "wg3": _w3(Wg[:, hsl], HG, WTERMS["g"]),
            "wo3": _wo3(Wo[cs, :]),
            "nbg4": np.tile(np.exp(-bg[None, hsl]), (C, 1)).astype(np.float32),
            "cosr": cosr, "sinr": sinr,
            "maskT": maskT, "ident": ident,
        })
    return in_maps


def kernel(x, Wq, Wk, Wv, Wg, bg, Wo, bo, _trace=False, **cfg):
    from concourse.bass_utils import run_bass_kernel_spmd
    nc = _get_nc(**cfg)
    in_maps = make_in_maps(x, Wq, Wk, Wv, Wg, bg, Wo, bo)
    res = run_bass_kernel_spmd(nc, in_maps, core_ids=list(range(NCORES)),
                               trace=_trace)
    out = np.zeros((B, L, HID), np.float32)
    for core in range(NCORES):
        b = core // 4
        out[b] += res.results[core]["out"].astype(np.float32)
    out += np.asarray(bo, np.float32)[None, None, :]
    kernel._last_results = res
    return out
